# revision 19
# baseline (speedup 1.0000x reference)
"""Trainium2 Bass kernel for a GNN attention block (8 NeuronCores, SPMD).

Model (per reference):
    K,Q,V = (x@Wk+bk, x@Wq+bq, x@Wv+bv) reshaped to (N, H, 64)
    att[e,h] = exp(Q[recv_e,h] . K[send_e,h] / 8 + const)
    out[n]   = (segment_sum(att * V[send], recv) / segment_sum(att, recv)) @ Wff + bff
The global-max shift in the reference cancels in the normalization, so a fixed
shift (-3) is used instead; results agree to fp rounding.

Sharding: receiver-node parallel. Core c owns a contiguous range of receiver
nodes; all edges into that range are processed there, so segment sums are
core-local. Each core projects K/V for its own node shard, the shards are
AllGathered, and per-edge K|V rows are fetched with per-chunk indirect
(gather) DMAs (128 rows per call — one row per SBUF partition, the form the
hardware descriptor generator supports). Q rows are expanded per edge on the
TensorEngine with a host-built one-hot matmul; the same one-hot computes the
segment sums (A^T @ U). The host does integer index bookkeeping only — all
floating-point math runs on the NeuronCores.
"""

import math
import os
os.environ.setdefault("JAX_COMPILATION_CACHE_DIR", "/root/.cache/jax_neff")
import numpy as np

import concourse.bass as bass
import concourse.bacc as bacc
import concourse.mybir as mybir
import concourse.tile as tile
from concourse.tile_rust import add_dep_helper
from concourse.bass_utils import run_bass_kernel_spmd

NCORES = 8
P = 128
FP16 = mybir.dt.float16
FP32 = mybir.dt.float32
I32 = mybir.dt.int32

_NC_CACHE = {}
# Phase-C structure: "pipe" pipelines per 128-edge chunk (better engine
# overlap); "tile" batches per 128-node tile. Both sim-validated.
PHASE_C = "pipe"


def _build(N, D, NT, C, NPC, has_bv, has_bkq=True, has_bff=True,
           profile_1core=False, phase_c=None):
    """Build the SPMD Bacc graph. NT: 128-node tiles per core; C: edge chunks
    (of 128) per tile; NPC = NT*128 padded nodes per core."""
    if phase_c is None:
        phase_c = PHASE_C
    H = 8
    DH = D // H          # 64
    ND = D // P          # 4 chunks of the feature dim
    KVFULL_ROWS = NCORES * NPC

    nc = bacc.Bacc("TRN2", target_bir_lowering=False,
                   num_devices=1 if profile_1core else NCORES)

    xT = nc.declare_dram_parameter("xT", [D, NPC], FP16, isOutput=False)
    wq = nc.declare_dram_parameter("wq", [D, D], FP16, isOutput=False)
    wk = nc.declare_dram_parameter("wk", [D, D], FP16, isOutput=False)
    wv = nc.declare_dram_parameter("wv", [D, D], FP16, isOutput=False)
    wff = nc.declare_dram_parameter("wff", [D, D], FP16, isOutput=False)
    bq_rep = nc.declare_dram_parameter("bq_rep", [P, D], FP16, isOutput=False)
    bk_rep = nc.declare_dram_parameter("bk_rep", [P, D], FP16, isOutput=False)
    bv_rep = nc.declare_dram_parameter("bv_rep", [P, D], FP16, isOutput=False)
    bff_rep = nc.declare_dram_parameter("bff_rep", [P, D], FP32, isOutput=False)
    ident = nc.declare_dram_parameter("ident", [P, P], FP16, isOutput=False)
    kv_idx = nc.declare_dram_parameter("kv_idx", [P, NT * C], I32, isOutput=False)
    amat = nc.declare_dram_parameter("amat", [P, NT * C, P], FP16, isOutput=False)
    amatT = nc.declare_dram_parameter("amatT", [P, NT * C, P], FP16, isOutput=False)
    out = nc.declare_dram_parameter("out", [NPC, D], FP32, isOutput=True)

    with tile.TileContext(nc) as tc:
        with (
            tc.tile_pool(name="dram", bufs=1, space="DRAM") as dram,
            tc.tile_pool(name="const", bufs=1) as cpool,
            tc.tile_pool(name="proj", bufs=2) as proj,
            tc.tile_pool(name="edge", bufs=2) as edge,
            tc.tile_pool(name="ps512", bufs=4, space="PSUM") as ps512,
            tc.tile_pool(name="psmall", bufs=2, space="PSUM") as psmall,
        ):
            kv_shard = dram.tile([NPC, 2 * D], FP16)
            kv_full = dram.tile([KVFULL_ROWS, 2 * D], FP16, addr_space="Shared")

            # ---- persistent constants in SBUF ----
            xt_sb = []
            for d in range(ND):
                t = cpool.tile([P, NPC], FP16, tag=f"xt{d}")
                nc.sync.dma_start(t[:], xT[d * P:(d + 1) * P, :])
                xt_sb.append(t)
            w_sb = {}
            for name, wt in (("q", wq), ("k", wk), ("v", wv), ("f", wff)):
                t = cpool.tile([P, ND, D], FP16, tag=f"w{name}")
                nc.sync.dma_start(t[:], wt[:].rearrange("(a p) n -> p a n", p=P))
                w_sb[name] = t
            bq_sb = cpool.tile([P, D], FP16, tag="bq")
            nc.sync.dma_start(bq_sb[:], bq_rep[:])
            bk_sb = cpool.tile([P, D], FP16, tag="bk")
            nc.sync.dma_start(bk_sb[:], bk_rep[:])
            bv_sb = cpool.tile([P, D], FP16, tag="bv")
            nc.sync.dma_start(bv_sb[:], bv_rep[:])
            bff_sb = cpool.tile([P, D], FP32, tag="bff")
            nc.sync.dma_start(bff_sb[:], bff_rep[:])
            id_sb = cpool.tile([P, P], FP16, tag="ident")
            nc.sync.dma_start(id_sb[:], ident[:])
            kvidx_sb = cpool.tile([P, NT * C], I32, tag="kvidx")
            nc.sync.dma_start(kvidx_sb[:], kv_idx[:])
            expbias_sb = cpool.tile([P, 1], FP32, tag="expbias")
            nc.gpsimd.memset(expbias_sb[:], -3.0)
            eps_sb = cpool.tile([P, 1], FP32, tag="eps")
            nc.gpsimd.memset(eps_sb[:], 1e-30)
            q_all = cpool.tile([P, NT, D], FP16, tag="qall")

            # ---- phase A: K/Q/V projections for this core's node shard ----
            kv_dmas = []
            for t in range(NT):
                pk = ps512.tile([P, D], FP32, tag="p512")
                pq = ps512.tile([P, D], FP32, tag="p512")
                pv = ps512.tile([P, D], FP32, tag="p512")
                for d in range(ND):
                    lhs = xt_sb[d][:, t * P:(t + 1) * P]
                    st, sp = d == 0, d == ND - 1
                    nc.tensor.matmul(pk[:], lhs, w_sb["k"][:, d, :], start=st, stop=sp)
                    nc.tensor.matmul(pq[:], lhs, w_sb["q"][:, d, :], start=st, stop=sp)
                    nc.tensor.matmul(pv[:], lhs, w_sb["v"][:, d, :], start=st, stop=sp)
                kv_sb = proj.tile([P, 2 * D], FP16, tag="kv")
                q_sb = q_all[:, t, :]
                if has_bkq or has_bv:
                    nc.vector.tensor_tensor(kv_sb[:, 0:D], pk[:], bk_sb[:], op=mybir.AluOpType.add)
                    nc.vector.tensor_tensor(kv_sb[:, D:2 * D], pv[:], bv_sb[:], op=mybir.AluOpType.add)
                    nc.vector.tensor_tensor(q_sb, pq[:], bq_sb[:], op=mybir.AluOpType.add)
                else:
                    nc.vector.tensor_copy(kv_sb[:, 0:D], pk[:])
                    nc.vector.tensor_copy(kv_sb[:, D:2 * D], pv[:])
                    nc.vector.tensor_copy(q_sb, pq[:])
                d1 = nc.sync.dma_start(kv_shard[t * P:(t + 1) * P, :], kv_sb[:])
                kv_dmas.append(d1)

            # ---- phase B: AllGather the K|V shard ----
            if profile_1core:
                # TimelineSim cannot model collectives; stand in a DMA copy so
                # the dependency structure stays the same.
                coll = nc.sync.dma_start(kv_full[0:NPC, :], kv_shard[:])
            else:
                coll = nc.gpsimd.collective_compute(
                    "AllGather",
                    mybir.AluOpType.bypass,
                    replica_groups=[list(range(NCORES))],
                    ins=[kv_shard.opt()],
                    outs=[kv_full.opt()],
                )
            for d1 in kv_dmas:
                add_dep_helper(coll.ins, d1.ins, reason="collective after shard write")

            # ---- phase C helpers ----
            def _tail(t, pagg, pssum):
                """normalize, bias, transpose, FF, store — per 128-node tile."""
                ssum = edge.tile([P, H], FP32, tag="ssum")
                nc.scalar.add(ssum[:], pssum[:], eps_sb[:])
                recip = edge.tile([P, H], FP32, tag="recip")
                nc.vector.reciprocal(recip[:], ssum[:])
                aggn = edge.tile([P, D], FP16, tag="aggn")
                nc.vector.tensor_tensor(
                    aggn[:].rearrange("p (h d) -> p h d", h=H),
                    pagg[:].rearrange("p (h d) -> p h d", h=H),
                    recip[:].unsqueeze(2).broadcast_to([P, H, DH]),
                    op=mybir.AluOpType.mult)
                if has_bv:
                    mask = edge.tile([P, H], FP16, tag="mask")
                    nc.scalar.sign(mask[:], pssum[:])
                    bvm = edge.tile([P, D], FP16, tag="bvm")
                    nc.vector.tensor_tensor(
                        bvm[:].rearrange("p (h d) -> p h d", h=H),
                        bv_sb[:].rearrange("p (h d) -> p h d", h=H),
                        mask[:].unsqueeze(2).broadcast_to([P, H, DH]),
                        op=mybir.AluOpType.mult)
                    nc.vector.tensor_tensor(aggn[:], aggn[:], bvm[:], op=mybir.AluOpType.add)

                aggnT = edge.tile([P, ND, P], FP16, tag="aggnT")
                for k in range(ND):
                    ptr = psmall.tile([P, P], FP16, tag="ptr")
                    nc.tensor.transpose(ptr[:], aggn[:, k * P:(k + 1) * P], id_sb[:])
                    nc.vector.tensor_copy(aggnT[:, k, :], ptr[:])
                pout = ps512.tile([P, D], FP32, tag="p512")
                for k in range(ND):
                    nc.tensor.matmul(pout[:], aggnT[:, k, :], w_sb["f"][:, k, :],
                                     start=(k == 0), stop=(k == ND - 1))
                out_sb = edge.tile([P, D], FP32, tag="outsb")
                if has_bff:
                    nc.vector.tensor_tensor(out_sb[:], pout[:], bff_sb[:], op=mybir.AluOpType.add)
                else:
                    nc.vector.tensor_copy(out_sb[:], pout[:])
                nc.sync.dma_start(out[t * P:(t + 1) * P, :], out_sb[:])

            def _gather_chunk(t, j, dest):
                g = nc.gpsimd.indirect_dma_start(
                    out=dest, out_offset=None, in_=kv_full[:],
                    in_offset=bass.IndirectOffsetOnAxis(
                        ap=kvidx_sb[:, t * C + j:t * C + j + 1], axis=0),
                )
                add_dep_helper(g.ins, coll.ins, reason="gather after allgather")

            # ---- phase C: per-tile edge processing + aggregation + FF ----
            for t in range(NT):
                a_sb = edge.tile([P, C, P], FP16, tag="amat")
                nc.sync.dma_start(a_sb[:], amat[:, t * C:(t + 1) * C, :])
                at_sb = edge.tile([P, C, P], FP16, tag="amatT")
                nc.sync.dma_start(at_sb[:], amatT[:, t * C:(t + 1) * C, :])

                if phase_c == "pipe":
                    pagg = ps512.tile([P, D], FP32, tag="p512")
                    pssum = psmall.tile([P, H], FP32, tag="pssum")
                    for j in range(C):
                        kvg_j = edge.tile([P, 2 * D], FP16, tag="kvgj", bufs=4)
                        _gather_chunk(t, j, kvg_j[:])
                        pqg = ps512.tile([P, D], FP32, tag="p512")
                        nc.tensor.matmul(pqg[:], at_sb[:, j, :], q_all[:, t, :],
                                         start=True, stop=True)
                        qg_sb = edge.tile([P, D], FP16, tag="qgsb", bufs=3)
                        nc.scalar.copy(qg_sb[:], pqg[:])
                        qk_j = edge.tile([P, D], FP16, tag="qkj", bufs=3)
                        nc.vector.tensor_tensor(qk_j[:], qg_sb[:], kvg_j[:, 0:D],
                                                op=mybir.AluOpType.mult)
                        attsum_j = edge.tile([P, H], FP32, tag="attsj", bufs=3)
                        nc.vector.tensor_reduce(
                            attsum_j[:], qk_j[:].rearrange("p (h d) -> p h d", h=H),
                            axis=mybir.AxisListType.X, op=mybir.AluOpType.add,
                        )
                        att8_j = edge.tile([P, H], FP16, tag="att8j", bufs=3)
                        nc.scalar.activation(att8_j[:], attsum_j[:],
                                             mybir.ActivationFunctionType.Exp,
                                             bias=expbias_sb[:],
                                             scale=1.0 / math.sqrt(DH))
                        e512_j = edge.tile([P, D], FP16, tag="e512j", bufs=3)
                        nc.scalar.activation(
                            e512_j[:].rearrange("p (h d) -> p h d", h=H),
                            attsum_j[:].unsqueeze(2).broadcast_to([P, H, DH]),
                            mybir.ActivationFunctionType.Exp,
                            bias=expbias_sb[:], scale=1.0 / math.sqrt(DH))
                        u_j = edge.tile([P, D], FP16, tag="uj", bufs=3)
                        nc.vector.tensor_tensor(u_j[:], kvg_j[:, D:2 * D], e512_j[:],
                                                op=mybir.AluOpType.mult)
                        st, sp = j == 0, j == C - 1
                        nc.tensor.matmul(pagg[:], a_sb[:, j, :], u_j[:], start=st, stop=sp)
                        nc.tensor.matmul(pssum[:], a_sb[:, j, :], att8_j[:], start=st, stop=sp)
                    _tail(t, pagg, pssum)
                    continue

                # phase_c == "tile": batched per-tile variant
                kv_g = edge.tile([P, C, 2 * D], FP16, tag="kvg")
                for j in range(C):
                    _gather_chunk(t, j, kv_g[:, j, :])
                qk = edge.tile([P, C, D], FP16, tag="qk")
                for j in range(C):
                    pqg = ps512.tile([P, D], FP32, tag="p512")
                    nc.tensor.matmul(pqg[:], at_sb[:, j, :], q_all[:, t, :],
                                     start=True, stop=True)
                    nc.vector.tensor_tensor(qk[:, j, :], pqg[:], kv_g[:, j, 0:D],
                                            op=mybir.AluOpType.mult)
                attsum = edge.tile([P, C, H], FP32, tag="attsum")
                nc.vector.tensor_reduce(
                    attsum[:], qk[:].rearrange("p c (h d) -> p c h d", h=H),
                    axis=mybir.AxisListType.X, op=mybir.AluOpType.add,
                )
                att8 = edge.tile([P, C, H], FP16, tag="att8")
                nc.scalar.activation(att8[:], attsum[:], mybir.ActivationFunctionType.Exp,
                                     bias=expbias_sb[:], scale=1.0 / math.sqrt(DH))
                exp512 = edge.tile([P, C, D], FP16, tag="exp512")
                nc.scalar.activation(
                    exp512[:].rearrange("p c (h d) -> p c h d", h=H),
                    attsum[:].unsqueeze(3).broadcast_to([P, C, H, DH]),
                    mybir.ActivationFunctionType.Exp,
                    bias=expbias_sb[:], scale=1.0 / math.sqrt(DH))
                u = edge.tile([P, C, D], FP16, tag="u")
                nc.vector.tensor_tensor(u[:], kv_g[:, :, D:2 * D], exp512[:],
                                        op=mybir.AluOpType.mult)
                pagg = ps512.tile([P, D], FP32, tag="p512")
                pssum = psmall.tile([P, H], FP32, tag="pssum")
                for j in range(C):
                    st, sp = j == 0, j == C - 1
                    nc.tensor.matmul(pagg[:], a_sb[:, j, :], u[:, j, :], start=st, stop=sp)
                    nc.tensor.matmul(pssum[:], a_sb[:, j, :], att8[:, j, :], start=st, stop=sp)
                _tail(t, pagg, pssum)

    nc.finalize()
    return nc


def _prep(inputs):
    """Host-side sharding / index bookkeeping. Returns (meta, in_maps)."""
    x = np.asarray(inputs["x"], np.float32)
    edge_index = np.asarray(inputs["edge_index"]).astype(np.int64)
    N, D = x.shape
    M = edge_index.shape[1]
    H = 8
    assert D % P == 0

    npc = (N + NCORES - 1) // NCORES          # nominal nodes per core
    NT = (npc + P - 1) // P
    NPC = NT * P
    NBINS = NCORES * NT

    senders, receivers = edge_index[0], edge_index[1]

    # Assign nodes to (core, tile, slot) by first-fit-decreasing bin packing on
    # in-degree: each 128-node tile gets at most ~6*128 edges, so the per-tile
    # edge-chunk count C (which sizes every gather/matmul loop) is minimized.
    # Pure host-side index bookkeeping; the device graph is unchanged.
    deg = np.bincount(receivers, minlength=N).astype(np.int64)
    node_order = np.argsort(-deg, kind="stable")
    bin_edges = np.zeros(NBINS, np.int64)
    bin_nodes = np.zeros(NBINS, np.int64)
    bin_of = np.empty(N, np.int64)
    slot_of = np.empty(N, np.int64)
    # LPT: place each node (descending degree) into the least-loaded bin with
    # node room — minimizes the max per-tile edge count, hence C.
    for n in node_order:
        cand = np.where(bin_nodes < P)[0]
        b = int(cand[np.argmin(bin_edges[cand])])
        bin_of[n] = b
        slot_of[n] = bin_nodes[b]
        bin_edges[b] += int(deg[n])
        bin_nodes[b] += 1
    core_node = bin_of // NT                  # per node
    tile_node = bin_of % NT
    row_node = tile_node * P + slot_of        # row within the core's NPC block

    core_of = core_node[receivers]
    tile_of = tile_node[receivers]
    group = bin_of[receivers]
    order = np.argsort(group, kind="stable")
    g_sorted = group[order]
    counts = np.bincount(g_sorted, minlength=NBINS)
    C = max(1, int(math.ceil(counts.max() / P)))

    offs = np.zeros(NBINS, np.int64)
    np.cumsum(counts[:-1], out=offs[1:])
    slot = np.arange(M) - offs[g_sorted]       # edge slot within tile group
    p_of = slot % P
    j_of = slot // P

    s_sorted = senders[order]
    send_row = (core_node[s_sorted] * NPC + row_node[s_sorted]).astype(np.int64)
    ncol_sorted = slot_of[receivers][order]    # one-hot col in tile

    kv_idx = np.zeros((NCORES, P, NT * C), np.int32)
    amat = np.zeros((NCORES, P, NT * C, P), np.float16)
    c_sorted = core_of[order]
    t_sorted = tile_of[order]
    col = t_sorted * C + j_of
    kv_idx[c_sorted, p_of, col] = send_row.astype(np.int32)
    amat[c_sorted, p_of, col, ncol_sorted] = np.float16(1.0)
    amatT = np.ascontiguousarray(amat.transpose(0, 3, 2, 1))

    wq = np.asarray(inputs["Wq"], np.float32).astype(np.float16)
    wk = np.asarray(inputs["Wk"], np.float32).astype(np.float16)
    wv = np.asarray(inputs["Wv"], np.float32).astype(np.float16)
    wff = np.asarray(inputs["Wff"], np.float32).astype(np.float16)
    bq = np.asarray(inputs["bq"], np.float32)
    bk = np.asarray(inputs["bk"], np.float32)
    bv = np.asarray(inputs["bv"], np.float32)
    bff = np.asarray(inputs["bff"], np.float32)
    has_bv = bool(np.any(bv != 0))
    has_bkq = bool(np.any(bq != 0) or np.any(bk != 0) or has_bv)
    has_bff = bool(np.any(bff != 0))

    bq_rep = np.broadcast_to(bq.astype(np.float16), (P, D)).copy()
    bk_rep = np.broadcast_to(bk.astype(np.float16), (P, D)).copy()
    bv_rep = np.broadcast_to(bv.astype(np.float16), (P, D)).copy()
    bff_rep = np.broadcast_to(bff, (P, D)).copy()
    ident = np.eye(P, dtype=np.float16)

    in_maps = []
    x16 = x.astype(np.float16)
    for c in range(NCORES):
        sel = np.where(core_node == c)[0]
        xs = np.zeros((NPC, D), np.float16)
        xs[row_node[sel]] = x16[sel]
        in_maps.append({
            "xT": np.ascontiguousarray(xs.T),
            "wq": wq, "wk": wk, "wv": wv, "wff": wff,
            "bq_rep": bq_rep, "bk_rep": bk_rep, "bv_rep": bv_rep,
            "bff_rep": bff_rep, "ident": ident,
            "kv_idx": kv_idx[c], "amat": amat[c], "amatT": amatT[c],
        })
    meta = dict(N=N, D=D, M=M, H=H, npc=npc, NT=NT, C=C, NPC=NPC, has_bv=has_bv,
                has_bkq=has_bkq, has_bff=has_bff)
    meta["core_node"] = core_node
    meta["row_node"] = row_node
    return meta, in_maps


def _get_nc(meta):
    key = (meta["N"], meta["D"], meta["NT"], meta["C"], meta["NPC"], meta["has_bv"],
           meta["has_bkq"], meta["has_bff"], PHASE_C)
    if key not in _NC_CACHE:
        _NC_CACHE[key] = _build(meta["N"], meta["D"], meta["NT"], meta["C"],
                                meta["NPC"], meta["has_bv"],
                                has_bkq=meta["has_bkq"], has_bff=meta["has_bff"])
    return _NC_CACHE[key]


def kernel(**inputs):
    meta, in_maps = _prep(inputs)
    nc = _get_nc(meta)
    res = run_bass_kernel_spmd(nc, in_maps, list(range(NCORES)))
    return _assemble(meta, [r["out"] for r in res.results])


def kernel_traced(**inputs):
    """Like kernel() but also returns the BassKernelResults (profiling, if
    available in the environment)."""
    meta, in_maps = _prep(inputs)
    nc = _get_nc(meta)
    try:
        res = run_bass_kernel_spmd(nc, in_maps, list(range(NCORES)), trace=True)
    except Exception:
        res = run_bass_kernel_spmd(nc, in_maps, list(range(NCORES)))
    return _assemble(meta, [r["out"] for r in res.results]), res


def _assemble(meta, outs):
    N, D = meta["N"], meta["D"]
    core_node, row_node = meta["core_node"], meta["row_node"]
    full = np.empty((N, D), np.float32)
    for c in range(NCORES):
        sel = np.where(core_node == c)[0]
        full[sel] = outs[c][row_node[sel]]
    return full
